# revision 10
# baseline (speedup 1.0000x reference)
"""Masked-copy kernel for nn_CompactExpandModule on 8 Trainium2 NeuronCores.

out[b, s] = input_embeddings[b, s] if token_ids[b, s] in keep_token_ids else 0

keep_token_ids is a contiguous range (arange(16000) per the problem spec), so
membership is a single compare against a threshold. Sharding is pure data
parallel: batch b -> core b (B == n_cores == 8).

Strategy (sparse gather): ~50% of rows are masked, so instead of streaming all
16 MiB of embeddings through SBUF and multiplying by the mask (DMA-fabric bound
at ~433 GB/s for 33.5 MB -> ~78 us + overheads), we:
  1. load token_ids, compute idx[r] = r if keep else r + 8192 (DVE),
  2. pre-zero the SBUF tiles (DVE memset, overlapped),
  3. indirect-gather ONLY the kept rows from HBM (idx > bounds_check=4095 are
     silently skipped by the DGE), landing them at their natural tile slots,
  4. dense-store every tile to the output.
HBM/fabric traffic drops to ~8.4 MB read + 16.8 MB write = 25.2 MB -> ~58 us.

Row layout: partition p owns rows p*32..p*32+31; tile t covers per-partition
columns [t*C, (t+1)*C). This makes token loads, iota (p*32+j), gathers, and
stores all share one indexing scheme with contiguous per-partition DMA chunks.

Written in raw Bass (explicit semaphores): the walrus build in this container
encodes at most ONE sync wait per instruction, which rules out the Tile
framework's aggregated multi-wait drains. Raw `wait_ge` emits standalone
single-wait instructions. Per-tile gather semaphores (not one cumulative sem)
because the 16 SDMA engines drain with skew: a cumulative threshold can be
reached before a lagging engine has landed tile t's data.
"""

import sys

if "/opt/trn_rl_repo" not in sys.path:
    sys.path.insert(0, "/opt/trn_rl_repo")

import contextlib

import numpy as np

import concourse.bass as bass
import concourse.mybir as mybir
from concourse.bass_utils import run_bass_kernel_spmd

B, S, D = 8, 4096, 1024
P = 128            # SBUF partitions
CPT = S // P       # 32 rows per partition total
NT = 8             # tiles per core
C = CPT // NT      # 4 rows per partition per tile -> 2 MiB tiles
N_CORES = 8
OOB_BUMP = 8192    # added to masked row indices; > bounds_check -> skipped

_program_cache: dict[tuple, bass.Bass] = {}


def _install_ntff_hook():
    """Register the axon NTFF profile hook that this image's boot skipped
    (its `antenv` package lacks `axon_hooks`). Mirrors trn_boot.py's
    `_ntff_profile_via_ctypes` against /opt/axon/libaxon_pjrt.so."""
    try:
        from antenv.axon_hooks import get_axon_ntff_profile_hook  # noqa: F401

        return True
    except ImportError:
        pass
    import ctypes
    import types

    try:
        lib = ctypes.CDLL("/opt/axon/libaxon_pjrt.so")
    except OSError:
        return False
    if not hasattr(lib, "axon_start_nrt_profile"):
        return False
    lib.axon_start_nrt_profile.argtypes = [
        ctypes.POINTER(ctypes.c_int64),
        ctypes.c_size_t,
    ]
    lib.axon_start_nrt_profile.restype = ctypes.c_int64
    lib.axon_stop_nrt_profile.argtypes = [ctypes.c_char_p]
    lib.axon_stop_nrt_profile.restype = ctypes.c_int64

    @contextlib.contextmanager
    def _hook(output_dir, device_ids):
        import jax

        jax.devices()
        if device_ids:
            ids = (ctypes.c_int64 * len(device_ids))(*device_ids)
            rc = lib.axon_start_nrt_profile(ids, len(device_ids))
        else:
            rc = lib.axon_start_nrt_profile(None, 0)
        if rc != 0:
            raise RuntimeError(f"axon_start_nrt_profile rc={rc}")
        try:
            yield
        finally:
            n = lib.axon_stop_nrt_profile(str(output_dir).encode())
            print(f"profile: {n} file(s) written to {output_dir}", file=sys.stderr)

    import antenv

    mod = types.ModuleType("antenv.axon_hooks")
    _state = {"hook": _hook}
    mod.set_axon_ntff_profile_hook = lambda h: _state.__setitem__("hook", h)
    mod.get_axon_ntff_profile_hook = lambda: _state["hook"]
    sys.modules["antenv.axon_hooks"] = mod
    antenv.axon_hooks = mod
    return True


def _build_program(hi: int, prezero: bool = True, mode: str = "gather",
                   use_bc: bool = True) -> bass.Bass:
    """One-core program: out = emb * (tok < hi), via sparse row gather.

    Engines: sync (SP/HWDGE) loads tok + dense-stores tiles; gpsimd (SWDGE)
    iota + indirect gathers; vector (DVE) computes idx + memsets tiles.

    mode='dense_gp' replaces the indirect gathers with plain dense loads
    (debug: output is then an unmasked copy). use_bc=False drops the
    bounds_check register (debug: OOB indices then error instead of skip).
    """
    key = (hi, prezero, mode, use_bc)
    if key in _program_cache:
        return _program_cache[key]

    nc = bass.Bass()
    emb = nc.declare_dram_parameter("emb", [S, D], mybir.dt.float32, isOutput=False)
    tok = nc.declare_dram_parameter("tok", [S], mybir.dt.int32, isOutput=False)
    out = nc.declare_dram_parameter("out", [S, D], mybir.dt.float32, isOutput=True)

    # row(p, j) = p*CPT + j; tile t is per-partition columns [t*C, (t+1)*C)
    tok_ap = tok[0:S].rearrange("(p j) -> p j", p=P)
    out_tiles = out[0:S, 0:D].rearrange("(p t c) d -> t p c d", p=P, t=NT, c=C)
    emb_all = emb[0:S, 0:D]  # gather source; offset must be 0
    emb_tiles = emb[0:S, 0:D].rearrange("(p t c) d -> t p c d", p=P, t=NT, c=C)

    with contextlib.ExitStack() as ctx:
        data = [
            ctx.enter_context(
                nc.sbuf_tensor(f"data{t}", [P, C, D], mybir.dt.float32)
            )
            for t in range(NT)
        ]
        tokbuf = ctx.enter_context(nc.sbuf_tensor("tokbuf", [P, CPT], mybir.dt.int32))
        idx = ctx.enter_context(nc.sbuf_tensor("idx", [P, CPT], mybir.dt.int32))
        oob = ctx.enter_context(nc.sbuf_tensor("oob", [P, CPT], mybir.dt.int32))

        tok_sem = ctx.enter_context(nc.semaphore("tok_sem"))
        oob_sem = ctx.enter_context(nc.semaphore("oob_sem"))
        iota_sem = ctx.enter_context(nc.semaphore("iota_sem"))
        idx_sem = ctx.enter_context(nc.semaphore("idx_sem"))
        zero_sem = ctx.enter_context(nc.semaphore("zero_sem"))
        gsems = [ctx.enter_context(nc.semaphore(f"gsem{t}")) for t in range(NT)]
        store_sem = ctx.enter_context(nc.semaphore("store_sem"))
        block = ctx.enter_context(nc.Block())

        gather_incs = 16 * C if mode == "gather_col" else 16

        @block.sync
        def _(sync: bass.BassEngine):
            sync.dma_start(out=tokbuf[:], in_=tok_ap).then_inc(tok_sem, 16)
            for t in range(NT):
                sync.wait_ge(gsems[t], gather_incs)
                sync.dma_start(out=out_tiles[t], in_=data[t][:]).then_inc(
                    store_sem, 16
                )
            sync.wait_ge(store_sem, 16 * NT)

        @block.gpsimd
        def _(gpsimd: bass.BassEngine):
            # idx[p, j] = p*CPT + j (the global row index)
            nc.gpsimd.iota(
                idx[:], pattern=[[1, CPT]], base=0, channel_multiplier=CPT
            ).then_inc(iota_sem, 1)
            gpsimd.wait_ge(idx_sem, 1)
            for t in range(NT):
                if prezero:
                    gpsimd.wait_ge(zero_sem, t + 1)
                if mode == "dense_gp":
                    gpsimd.dma_start(
                        out=data[t][:], in_=emb_tiles[t]
                    ).then_inc(gsems[t], 16)
                elif mode == "gather_col":
                    # one gather per column: [P, 1] indices, 2D [P, D] out —
                    # the exact shape tile_scatter_add exercises.
                    for c in range(C):
                        j = t * C + c
                        nc.gpsimd.indirect_dma_start(
                            out=data[t][:, c, :],
                            out_offset=None,
                            in_=emb_all,
                            in_offset=bass.IndirectOffsetOnAxis(
                                ap=idx[:, j : j + 1], axis=0
                            ),
                            bounds_check=S - 1,
                            oob_is_err=False,
                        ).then_inc(gsems[t], 16)
                elif use_bc:
                    nc.gpsimd.indirect_dma_start(
                        out=data[t][:],
                        out_offset=None,
                        in_=emb_all,
                        in_offset=bass.IndirectOffsetOnAxis(
                            ap=idx[:, t * C : (t + 1) * C], axis=0
                        ),
                        bounds_check=S - 1,
                        oob_is_err=False,
                    ).then_inc(gsems[t], 16)
                else:
                    nc.gpsimd.indirect_dma_start(
                        out=data[t][:],
                        out_offset=None,
                        in_=emb_all,
                        in_offset=bass.IndirectOffsetOnAxis(
                            ap=idx[:, t * C : (t + 1) * C], axis=0
                        ),
                    ).then_inc(gsems[t], 16)

        @block.vector
        def _(vector: bass.BassEngine):
            if prezero:
                nc.vector.memset(data[0][:], 0.0).then_inc(zero_sem, 1)
            vector.wait_ge(tok_sem, 16)
            # oob = (tok >= hi) * OOB_BUMP
            nc.vector.tensor_scalar(
                out=oob[:], in0=tokbuf[:], scalar1=hi, scalar2=OOB_BUMP,
                op0=mybir.AluOpType.is_ge, op1=mybir.AluOpType.mult,
            ).then_inc(oob_sem, 1)
            # DVE pipelines; a same-engine RAW (oob write -> read) still
            # needs a semaphore (CoreSim race detector flags it otherwise).
            vector.wait_ge(oob_sem, 1)
            vector.wait_ge(iota_sem, 1)
            nc.vector.tensor_tensor(
                out=idx[:], in0=idx[:], in1=oob[:], op=mybir.AluOpType.add
            ).then_inc(idx_sem, 1)
            if prezero:
                for t in range(1, NT):
                    nc.vector.memset(data[t][:], 0.0).then_inc(zero_sem, 1)

    _program_cache[key] = nc
    return nc


def _keep_range(keep_token_ids: np.ndarray) -> tuple[int, int] | None:
    """If keep_token_ids is a contiguous integer range, return (lo, hi)."""
    k = np.asarray(keep_token_ids)
    if k.ndim != 1 or k.size == 0:
        return None
    lo = int(k.min())
    hi = int(k.max()) + 1
    if hi - lo == k.size and np.unique(k).size == k.size:
        return lo, hi
    return None


def kernel(input_embeddings, token_ids, keep_token_ids, _want_timing=False,
           _prezero=True):
    emb = np.ascontiguousarray(np.asarray(input_embeddings, dtype=np.float32))
    tok = np.ascontiguousarray(np.asarray(token_ids, dtype=np.int32))
    keep = np.asarray(keep_token_ids)
    assert emb.shape == (B, S, D) and tok.shape == (B, S)

    rng = _keep_range(keep)
    if rng is None or rng[0] != 0:
        # Keep-set is not arange(0, k) (not expected per spec): remap token
        # ids on the host so the device threshold compare still yields isin().
        tok = np.where(np.isin(tok, keep), np.int32(0), np.int32(1)).astype(np.int32)
        hi = 1
    else:
        hi = rng[1]

    if _want_timing:
        _want_timing = _install_ntff_hook()
    nc = _build_program(hi, prezero=_prezero, mode="gather_col")
    in_maps = [{"emb": emb[b], "tok": tok[b]} for b in range(B)]
    res = run_bass_kernel_spmd(
        nc, in_maps, list(range(N_CORES)), trace=bool(_want_timing)
    )
    out = np.stack([np.asarray(res.results[b]["out"]) for b in range(B)], axis=0)
    if _want_timing:
        return out, res.exec_time_ns
    return out


# revision 13
# speedup vs baseline: 1.0660x; 1.0660x over previous
"""Masked-copy kernel for nn_CompactExpandModule on 8 Trainium2 NeuronCores.

out[b, s] = input_embeddings[b, s] if token_ids[b, s] in keep_token_ids else 0

keep_token_ids is a contiguous range (arange(16000) per the problem spec), so
membership is a single compare against a threshold. Sharding is pure data
parallel: batch b -> core b (B == n_cores == 8).

Strategy (sparse gather): ~50% of rows are masked, so instead of streaming all
16 MiB of embeddings through SBUF and multiplying by the mask (DMA-fabric bound
at ~433 GB/s for 33.5 MB -> ~78 us + overheads), we:
  1. load token_ids, compute idx[r] = r if keep else r + 8192 (DVE),
  2. pre-zero the SBUF tiles (DVE memset, overlapped),
  3. indirect-gather ONLY the kept rows from HBM (idx > bounds_check=4095 are
     silently skipped by the DGE), landing them at their natural tile slots,
  4. dense-store every tile to the output.
HBM/fabric traffic drops to ~8.4 MB read + 16.8 MB write = 25.2 MB -> ~58 us.

Row layout: partition p owns rows p*32..p*32+31; tile t covers per-partition
columns [t*C, (t+1)*C). This makes token loads, iota (p*32+j), gathers, and
stores all share one indexing scheme with contiguous per-partition DMA chunks.

Written in raw Bass (explicit semaphores): the walrus build in this container
encodes at most ONE sync wait per instruction, which rules out the Tile
framework's aggregated multi-wait drains. Raw `wait_ge` emits standalone
single-wait instructions. Per-tile gather semaphores (not one cumulative sem)
because the 16 SDMA engines drain with skew: a cumulative threshold can be
reached before a lagging engine has landed tile t's data.
"""

import sys

if "/opt/trn_rl_repo" not in sys.path:
    sys.path.insert(0, "/opt/trn_rl_repo")

import contextlib

import numpy as np

import concourse.bass as bass
import concourse.mybir as mybir
from concourse.bass_utils import run_bass_kernel_spmd

B, S, D = 8, 4096, 1024
P = 128            # SBUF partitions
CPT = S // P       # 32 rows per partition total
NT = 8             # tiles per core
C = CPT // NT      # 4 rows per partition per tile -> 2 MiB tiles
N_CORES = 8
OOB_BUMP = 8192    # added to masked row indices; > bounds_check -> skipped

_program_cache: dict[tuple, bass.Bass] = {}


def _install_ntff_hook():
    """Register the axon NTFF profile hook that this image's boot skipped
    (its `antenv` package lacks `axon_hooks`). Mirrors trn_boot.py's
    `_ntff_profile_via_ctypes` against /opt/axon/libaxon_pjrt.so."""
    try:
        from antenv.axon_hooks import get_axon_ntff_profile_hook  # noqa: F401

        return True
    except ImportError:
        pass
    import ctypes
    import types

    try:
        lib = ctypes.CDLL("/opt/axon/libaxon_pjrt.so")
    except OSError:
        return False
    if not hasattr(lib, "axon_start_nrt_profile"):
        return False
    lib.axon_start_nrt_profile.argtypes = [
        ctypes.POINTER(ctypes.c_int64),
        ctypes.c_size_t,
    ]
    lib.axon_start_nrt_profile.restype = ctypes.c_int64
    lib.axon_stop_nrt_profile.argtypes = [ctypes.c_char_p]
    lib.axon_stop_nrt_profile.restype = ctypes.c_int64

    @contextlib.contextmanager
    def _hook(output_dir, device_ids):
        import jax

        jax.devices()
        if device_ids:
            ids = (ctypes.c_int64 * len(device_ids))(*device_ids)
            rc = lib.axon_start_nrt_profile(ids, len(device_ids))
        else:
            rc = lib.axon_start_nrt_profile(None, 0)
        if rc != 0:
            raise RuntimeError(f"axon_start_nrt_profile rc={rc}")
        try:
            yield
        finally:
            n = lib.axon_stop_nrt_profile(str(output_dir).encode())
            print(f"profile: {n} file(s) written to {output_dir}", file=sys.stderr)

    import antenv

    mod = types.ModuleType("antenv.axon_hooks")
    _state = {"hook": _hook}
    mod.set_axon_ntff_profile_hook = lambda h: _state.__setitem__("hook", h)
    mod.get_axon_ntff_profile_hook = lambda: _state["hook"]
    sys.modules["antenv.axon_hooks"] = mod
    antenv.axon_hooks = mod
    return True


def _build_program(hi: int, prezero: bool = True, mode: str = "gather",
                   use_bc: bool = True) -> bass.Bass:
    """One-core program: out = emb * (tok < hi), via sparse row gather.

    Engines: sync (SP/HWDGE) loads tok + dense-stores tiles; gpsimd (SWDGE)
    iota + indirect gathers; vector (DVE) computes idx + memsets tiles.

    mode='dense_gp' replaces the indirect gathers with plain dense loads
    (debug: output is then an unmasked copy). use_bc=False drops the
    bounds_check register (debug: OOB indices then error instead of skip).
    """
    key = (hi, prezero, mode, use_bc)
    if key in _program_cache:
        return _program_cache[key]

    nc = bass.Bass()
    emb = nc.declare_dram_parameter("emb", [S, D], mybir.dt.float32, isOutput=False)
    tok = nc.declare_dram_parameter("tok", [S], mybir.dt.int32, isOutput=False)
    out = nc.declare_dram_parameter("out", [S, D], mybir.dt.float32, isOutput=True)

    # row(p, j) = p*CPT + j; tile t is per-partition columns [t*C, (t+1)*C)
    tok_ap = tok[0:S].rearrange("(p j) -> p j", p=P)
    out_tiles = out[0:S, 0:D].rearrange("(p t c) d -> t p c d", p=P, t=NT, c=C)
    emb_all = emb[0:S, 0:D]  # gather source; offset must be 0
    out_all = out[0:S, 0:D]  # scatter dest; offset must be 0
    emb_tiles = emb[0:S, 0:D].rearrange("(p t c) d -> t p c d", p=P, t=NT, c=C)

    with contextlib.ExitStack() as ctx:
        data = [
            ctx.enter_context(
                nc.sbuf_tensor(f"data{t}", [P, C, D], mybir.dt.float32)
            )
            for t in range(NT)
        ]
        tokbuf = ctx.enter_context(nc.sbuf_tensor("tokbuf", [P, CPT], mybir.dt.int32))
        idx = ctx.enter_context(nc.sbuf_tensor("idx", [P, CPT], mybir.dt.int32))
        oob = ctx.enter_context(nc.sbuf_tensor("oob", [P, CPT], mybir.dt.int32))

        tok_sem = ctx.enter_context(nc.semaphore("tok_sem"))
        oob_sem = ctx.enter_context(nc.semaphore("oob_sem"))
        iota_sem = ctx.enter_context(nc.semaphore("iota_sem"))
        idx_sem = ctx.enter_context(nc.semaphore("idx_sem"))
        zero_sem = ctx.enter_context(nc.semaphore("zero_sem"))
        gsems = [ctx.enter_context(nc.semaphore(f"gsem{t}")) for t in range(NT)]
        store_sem = ctx.enter_context(nc.semaphore("store_sem"))
        block = ctx.enter_context(nc.Block())

        gather_incs = 16 * C if mode == "gather_col" else 16

        @block.sync
        def _(sync: bass.BassEngine):
            sync.dma_start(out=tokbuf[:], in_=tok_ap).then_inc(tok_sem, 16)
            if mode == "scatter":
                # dense loads; the sparse side is the scatter on gpsimd
                for t in range(NT):
                    sync.dma_start(out=data[t][:], in_=emb_tiles[t]).then_inc(
                        gsems[t], 16
                    )
                return
            for t in range(NT):
                sync.wait_ge(gsems[t], gather_incs)
                sync.dma_start(out=out_tiles[t], in_=data[t][:]).then_inc(
                    store_sem, 16
                )
            sync.wait_ge(store_sem, 16 * NT)

        if mode == "scatter":

            @block.gpsimd
            def _(gpsimd: bass.BassEngine):
                # idx[p, j] = p*CPT + j (the global row index)
                nc.gpsimd.iota(
                    idx[:], pattern=[[1, CPT]], base=0, channel_multiplier=CPT
                ).then_inc(iota_sem, 1)
                gpsimd.wait_ge(idx_sem, 1)
                for t in range(NT):
                    gpsimd.wait_ge(gsems[t], 16)  # tile t load landed
                    for c in range(C):
                        j = t * C + c
                        nc.gpsimd.indirect_dma_start(
                            out=out_all,
                            out_offset=bass.IndirectOffsetOnAxis(
                                ap=idx[:, j : j + 1], axis=0
                            ),
                            in_=data[t][:, c, :],
                            in_offset=None,
                            bounds_check=S - 1,
                            oob_is_err=False,
                        ).then_inc(store_sem, 16)
                gpsimd.wait_ge(store_sem, 16 * NT * C)

            @block.vector
            def _(vector: bass.BassEngine):
                vector.wait_ge(tok_sem, 16)
                nc.vector.tensor_scalar(
                    out=oob[:], in0=tokbuf[:], scalar1=hi, scalar2=OOB_BUMP,
                    op0=mybir.AluOpType.is_ge, op1=mybir.AluOpType.mult,
                ).then_inc(oob_sem, 1)
                vector.wait_ge(oob_sem, 1)
                vector.wait_ge(iota_sem, 1)
                nc.vector.tensor_tensor(
                    out=idx[:], in0=idx[:], in1=oob[:], op=mybir.AluOpType.add
                ).then_inc(idx_sem, 1)

            _program_cache[key] = nc
            return nc

        @block.gpsimd
        def _(gpsimd: bass.BassEngine):
            # idx[p, j] = p*CPT + j (the global row index)
            nc.gpsimd.iota(
                idx[:], pattern=[[1, CPT]], base=0, channel_multiplier=CPT
            ).then_inc(iota_sem, 1)
            gpsimd.wait_ge(idx_sem, 1)
            for t in range(NT):
                if prezero:
                    gpsimd.wait_ge(zero_sem, t + 1)
                if mode == "dense_gp":
                    gpsimd.dma_start(
                        out=data[t][:], in_=emb_tiles[t]
                    ).then_inc(gsems[t], 16)
                elif mode == "gather_col":
                    # one gather per column: [P, 1] indices, 2D [P, D] out —
                    # the exact shape tile_scatter_add exercises.
                    for c in range(C):
                        j = t * C + c
                        nc.gpsimd.indirect_dma_start(
                            out=data[t][:, c, :],
                            out_offset=None,
                            in_=emb_all,
                            in_offset=bass.IndirectOffsetOnAxis(
                                ap=idx[:, j : j + 1], axis=0
                            ),
                            bounds_check=S - 1,
                            oob_is_err=False,
                        ).then_inc(gsems[t], 16)
                elif use_bc:
                    nc.gpsimd.indirect_dma_start(
                        out=data[t][:],
                        out_offset=None,
                        in_=emb_all,
                        in_offset=bass.IndirectOffsetOnAxis(
                            ap=idx[:, t * C : (t + 1) * C], axis=0
                        ),
                        bounds_check=S - 1,
                        oob_is_err=False,
                    ).then_inc(gsems[t], 16)
                else:
                    nc.gpsimd.indirect_dma_start(
                        out=data[t][:],
                        out_offset=None,
                        in_=emb_all,
                        in_offset=bass.IndirectOffsetOnAxis(
                            ap=idx[:, t * C : (t + 1) * C], axis=0
                        ),
                    ).then_inc(gsems[t], 16)

        @block.vector
        def _(vector: bass.BassEngine):
            if prezero:
                nc.vector.memset(data[0][:], 0.0).then_inc(zero_sem, 1)
            vector.wait_ge(tok_sem, 16)
            # oob = (tok >= hi) * OOB_BUMP
            nc.vector.tensor_scalar(
                out=oob[:], in0=tokbuf[:], scalar1=hi, scalar2=OOB_BUMP,
                op0=mybir.AluOpType.is_ge, op1=mybir.AluOpType.mult,
            ).then_inc(oob_sem, 1)
            # DVE pipelines; a same-engine RAW (oob write -> read) still
            # needs a semaphore (CoreSim race detector flags it otherwise).
            vector.wait_ge(oob_sem, 1)
            vector.wait_ge(iota_sem, 1)
            nc.vector.tensor_tensor(
                out=idx[:], in0=idx[:], in1=oob[:], op=mybir.AluOpType.add
            ).then_inc(idx_sem, 1)
            if prezero:
                for t in range(1, NT):
                    nc.vector.memset(data[t][:], 0.0).then_inc(zero_sem, 1)

    _program_cache[key] = nc
    return nc


def _keep_range(keep_token_ids: np.ndarray) -> tuple[int, int] | None:
    """If keep_token_ids is a contiguous integer range, return (lo, hi)."""
    k = np.asarray(keep_token_ids)
    if k.ndim != 1 or k.size == 0:
        return None
    lo = int(k.min())
    hi = int(k.max()) + 1
    if hi - lo == k.size and np.unique(k).size == k.size:
        return lo, hi
    return None


def kernel(input_embeddings, token_ids, keep_token_ids, _want_timing=False,
           _prezero=True):
    emb = np.ascontiguousarray(np.asarray(input_embeddings, dtype=np.float32))
    tok = np.ascontiguousarray(np.asarray(token_ids, dtype=np.int32))
    keep = np.asarray(keep_token_ids)
    assert emb.shape == (B, S, D) and tok.shape == (B, S)

    rng = _keep_range(keep)
    if rng is None or rng[0] != 0:
        # Keep-set is not arange(0, k) (not expected per spec): remap token
        # ids on the host so the device threshold compare still yields isin().
        tok = np.where(np.isin(tok, keep), np.int32(0), np.int32(1)).astype(np.int32)
        hi = 1
    else:
        hi = rng[1]

    if _want_timing:
        _want_timing = _install_ntff_hook()
    nc = _build_program(hi, prezero=_prezero, mode="scatter")
    in_maps = [{"emb": emb[b], "tok": tok[b]} for b in range(B)]
    res = run_bass_kernel_spmd(
        nc, in_maps, list(range(N_CORES)), trace=bool(_want_timing)
    )
    out = np.stack([np.asarray(res.results[b]["out"]) for b in range(B)], axis=0)
    if _want_timing:
        return out, res.exec_time_ns
    return out


# revision 16
# speedup vs baseline: 1.2289x; 1.1527x over previous
"""Masked-copy kernel for nn_CompactExpandModule on 8 Trainium2 NeuronCores.

out[b, s] = input_embeddings[b, s] if token_ids[b, s] in keep_token_ids else 0

keep_token_ids is a contiguous range (arange(16000) per the problem spec), so
membership is a single compare against a threshold. Sharding is pure data
parallel: batch b -> core b (B == n_cores == 8).

Strategy (sparse gather): ~50% of rows are masked, so instead of streaming all
16 MiB of embeddings through SBUF and multiplying by the mask (DMA-fabric bound
at ~433 GB/s for 33.5 MB -> ~78 us + overheads), we:
  1. load token_ids, compute idx[r] = r if keep else r + 8192 (DVE),
  2. pre-zero the SBUF tiles (DVE memset, overlapped),
  3. indirect-gather ONLY the kept rows from HBM (idx > bounds_check=4095 are
     silently skipped by the DGE), landing them at their natural tile slots,
  4. dense-store every tile to the output.
HBM/fabric traffic drops to ~8.4 MB read + 16.8 MB write = 25.2 MB -> ~58 us.

Row layout: partition p owns rows p*32..p*32+31; tile t covers per-partition
columns [t*C, (t+1)*C). This makes token loads, iota (p*32+j), gathers, and
stores all share one indexing scheme with contiguous per-partition DMA chunks.

Written in raw Bass (explicit semaphores): the walrus build in this container
encodes at most ONE sync wait per instruction, which rules out the Tile
framework's aggregated multi-wait drains. Raw `wait_ge` emits standalone
single-wait instructions. Per-tile gather semaphores (not one cumulative sem)
because the 16 SDMA engines drain with skew: a cumulative threshold can be
reached before a lagging engine has landed tile t's data.
"""

import sys

if "/opt/trn_rl_repo" not in sys.path:
    sys.path.insert(0, "/opt/trn_rl_repo")

import contextlib

import numpy as np

import concourse.bass as bass
import concourse.mybir as mybir
from concourse.bass_utils import run_bass_kernel_spmd

B, S, D = 8, 4096, 1024
P = 128            # SBUF partitions
CPT = S // P       # 32 rows per partition total
NT = 8             # tiles per core
C = CPT // NT      # 4 rows per partition per tile -> 2 MiB tiles
N_CORES = 8
OOB_BUMP = 8192    # added to masked row indices; > bounds_check -> skipped

_program_cache: dict[tuple, bass.Bass] = {}


def _install_ntff_hook():
    """Register the axon NTFF profile hook that this image's boot skipped
    (its `antenv` package lacks `axon_hooks`). Mirrors trn_boot.py's
    `_ntff_profile_via_ctypes` against /opt/axon/libaxon_pjrt.so."""
    try:
        from antenv.axon_hooks import get_axon_ntff_profile_hook  # noqa: F401

        return True
    except ImportError:
        pass
    import ctypes
    import types

    try:
        lib = ctypes.CDLL("/opt/axon/libaxon_pjrt.so")
    except OSError:
        return False
    if not hasattr(lib, "axon_start_nrt_profile"):
        return False
    lib.axon_start_nrt_profile.argtypes = [
        ctypes.POINTER(ctypes.c_int64),
        ctypes.c_size_t,
    ]
    lib.axon_start_nrt_profile.restype = ctypes.c_int64
    lib.axon_stop_nrt_profile.argtypes = [ctypes.c_char_p]
    lib.axon_stop_nrt_profile.restype = ctypes.c_int64

    @contextlib.contextmanager
    def _hook(output_dir, device_ids):
        import jax

        jax.devices()
        if device_ids:
            ids = (ctypes.c_int64 * len(device_ids))(*device_ids)
            rc = lib.axon_start_nrt_profile(ids, len(device_ids))
        else:
            rc = lib.axon_start_nrt_profile(None, 0)
        if rc != 0:
            raise RuntimeError(f"axon_start_nrt_profile rc={rc}")
        try:
            yield
        finally:
            n = lib.axon_stop_nrt_profile(str(output_dir).encode())
            print(f"profile: {n} file(s) written to {output_dir}", file=sys.stderr)

    import antenv

    mod = types.ModuleType("antenv.axon_hooks")
    _state = {"hook": _hook}
    mod.set_axon_ntff_profile_hook = lambda h: _state.__setitem__("hook", h)
    mod.get_axon_ntff_profile_hook = lambda: _state["hook"]
    sys.modules["antenv.axon_hooks"] = mod
    antenv.axon_hooks = mod
    return True


def _build_program(hi: int, prezero: bool = True, mode: str = "gather",
                   use_bc: bool = True) -> bass.Bass:
    """One-core program: out = emb * (tok < hi), via sparse row gather.

    Engines: sync (SP/HWDGE) loads tok + dense-stores tiles; gpsimd (SWDGE)
    iota + indirect gathers; vector (DVE) computes idx + memsets tiles.

    mode='dense_gp' replaces the indirect gathers with plain dense loads
    (debug: output is then an unmasked copy). use_bc=False drops the
    bounds_check register (debug: OOB indices then error instead of skip).
    """
    key = (hi, prezero, mode, use_bc)
    if key in _program_cache:
        return _program_cache[key]

    nc = bass.Bass()
    emb = nc.declare_dram_parameter("emb", [S, D], mybir.dt.float32, isOutput=False)
    tok = nc.declare_dram_parameter("tok", [S], mybir.dt.int32, isOutput=False)
    out = nc.declare_dram_parameter("out", [S, D], mybir.dt.float32, isOutput=True)

    # row(p, j) = p*CPT + j; tile t is per-partition columns [t*C, (t+1)*C)
    tok_ap = tok[0:S].rearrange("(p j) -> p j", p=P)
    out_tiles = out[0:S, 0:D].rearrange("(p t c) d -> t p c d", p=P, t=NT, c=C)
    emb_all = emb[0:S, 0:D]  # gather source; offset must be 0
    out_all = out[0:S, 0:D]  # scatter dest; offset must be 0
    emb_tiles = emb[0:S, 0:D].rearrange("(p t c) d -> t p c d", p=P, t=NT, c=C)

    with contextlib.ExitStack() as ctx:
        data = [
            ctx.enter_context(
                nc.sbuf_tensor(f"data{t}", [P, C, D], mybir.dt.float32)
            )
            for t in range(NT)
        ]
        tokbuf = ctx.enter_context(nc.sbuf_tensor("tokbuf", [P, CPT], mybir.dt.int32))
        idx = ctx.enter_context(nc.sbuf_tensor("idx", [P, CPT], mybir.dt.int32))
        oob = ctx.enter_context(nc.sbuf_tensor("oob", [P, CPT], mybir.dt.int32))

        tok_sem = ctx.enter_context(nc.semaphore("tok_sem"))
        oob_sem = ctx.enter_context(nc.semaphore("oob_sem"))
        if mode == "scatter":
            hsems = [
                ctx.enter_context(nc.semaphore(f"hsem{i}"))
                for i in range(NT * (C // 2))
            ]
        iota_sem = ctx.enter_context(nc.semaphore("iota_sem"))
        idx_sem = ctx.enter_context(nc.semaphore("idx_sem"))
        zero_sem = ctx.enter_context(nc.semaphore("zero_sem"))
        gsems = [ctx.enter_context(nc.semaphore(f"gsem{t}")) for t in range(NT)]
        store_sem = ctx.enter_context(nc.semaphore("store_sem"))
        block = ctx.enter_context(nc.Block())

        gather_incs = 16 * C if mode == "gather_col" else 16

        @block.sync
        def _(sync: bass.BassEngine):
            sync.dma_start(out=tokbuf[:], in_=tok_ap).then_inc(tok_sem, 16)
            if mode == "scatter":
                # Dense loads; the sparse side is the scatter on gpsimd.
                # Half-tile ops (2 rows/partition = 8 KiB descriptors): load
                # packets are then 2x the scatter's 4 KiB descriptors, so the
                # SDMA packet round-robin splits fabric ~2:1 load:scatter --
                # matching the 2:1 byte ratio so neither stream backlogs.
                for t in range(NT):
                    for h in range(C // 2):
                        sync.dma_start(
                            out=data[t][:, 2 * h : 2 * h + 2, :],
                            in_=emb_tiles[t][:, 2 * h : 2 * h + 2, :],
                        ).then_inc(hsems[t * (C // 2) + h], 16)
                return
            for t in range(NT):
                sync.wait_ge(gsems[t], gather_incs)
                sync.dma_start(out=out_tiles[t], in_=data[t][:]).then_inc(
                    store_sem, 16
                )
            sync.wait_ge(store_sem, 16 * NT)

        if mode == "scatter":

            @block.gpsimd
            def _(gpsimd: bass.BassEngine):
                # idx[p, j] = p*CPT + j (the global row index)
                nc.gpsimd.iota(
                    idx[:], pattern=[[1, CPT]], base=0, channel_multiplier=CPT
                ).then_inc(iota_sem, 1)
                bc_reg = nc.gpsimd.to_reg(S - 1)  # hoisted out of the loop
                gpsimd.wait_ge(idx_sem, 1)
                for t in range(NT):
                    for c in range(C):
                        if c % 2 == 0:  # half-tile (2 columns) landed
                            gpsimd.wait_ge(hsems[t * (C // 2) + c // 2], 16)
                        j = t * C + c
                        nc.gpsimd.indirect_dma_start(
                            out=out_all,
                            out_offset=bass.IndirectOffsetOnAxis(
                                ap=idx[:, j : j + 1], axis=0
                            ),
                            in_=data[t][:, c, :],
                            in_offset=None,
                            bounds_check=bc_reg,
                            oob_is_err=False,
                        ).then_inc(store_sem, 16)
                gpsimd.wait_ge(store_sem, 16 * NT * C)

            @block.vector
            def _(vector: bass.BassEngine):
                vector.wait_ge(tok_sem, 16)
                nc.vector.tensor_scalar(
                    out=oob[:], in0=tokbuf[:], scalar1=hi, scalar2=OOB_BUMP,
                    op0=mybir.AluOpType.is_ge, op1=mybir.AluOpType.mult,
                ).then_inc(oob_sem, 1)
                vector.wait_ge(oob_sem, 1)
                vector.wait_ge(iota_sem, 1)
                nc.vector.tensor_tensor(
                    out=idx[:], in0=idx[:], in1=oob[:], op=mybir.AluOpType.add
                ).then_inc(idx_sem, 1)

            _program_cache[key] = nc
            return nc

        @block.gpsimd
        def _(gpsimd: bass.BassEngine):
            # idx[p, j] = p*CPT + j (the global row index)
            nc.gpsimd.iota(
                idx[:], pattern=[[1, CPT]], base=0, channel_multiplier=CPT
            ).then_inc(iota_sem, 1)
            gpsimd.wait_ge(idx_sem, 1)
            for t in range(NT):
                if prezero:
                    gpsimd.wait_ge(zero_sem, t + 1)
                if mode == "dense_gp":
                    gpsimd.dma_start(
                        out=data[t][:], in_=emb_tiles[t]
                    ).then_inc(gsems[t], 16)
                elif mode == "gather_col":
                    # one gather per column: [P, 1] indices, 2D [P, D] out —
                    # the exact shape tile_scatter_add exercises.
                    for c in range(C):
                        j = t * C + c
                        nc.gpsimd.indirect_dma_start(
                            out=data[t][:, c, :],
                            out_offset=None,
                            in_=emb_all,
                            in_offset=bass.IndirectOffsetOnAxis(
                                ap=idx[:, j : j + 1], axis=0
                            ),
                            bounds_check=S - 1,
                            oob_is_err=False,
                        ).then_inc(gsems[t], 16)
                elif use_bc:
                    nc.gpsimd.indirect_dma_start(
                        out=data[t][:],
                        out_offset=None,
                        in_=emb_all,
                        in_offset=bass.IndirectOffsetOnAxis(
                            ap=idx[:, t * C : (t + 1) * C], axis=0
                        ),
                        bounds_check=S - 1,
                        oob_is_err=False,
                    ).then_inc(gsems[t], 16)
                else:
                    nc.gpsimd.indirect_dma_start(
                        out=data[t][:],
                        out_offset=None,
                        in_=emb_all,
                        in_offset=bass.IndirectOffsetOnAxis(
                            ap=idx[:, t * C : (t + 1) * C], axis=0
                        ),
                    ).then_inc(gsems[t], 16)

        @block.vector
        def _(vector: bass.BassEngine):
            if prezero:
                nc.vector.memset(data[0][:], 0.0).then_inc(zero_sem, 1)
            vector.wait_ge(tok_sem, 16)
            # oob = (tok >= hi) * OOB_BUMP
            nc.vector.tensor_scalar(
                out=oob[:], in0=tokbuf[:], scalar1=hi, scalar2=OOB_BUMP,
                op0=mybir.AluOpType.is_ge, op1=mybir.AluOpType.mult,
            ).then_inc(oob_sem, 1)
            # DVE pipelines; a same-engine RAW (oob write -> read) still
            # needs a semaphore (CoreSim race detector flags it otherwise).
            vector.wait_ge(oob_sem, 1)
            vector.wait_ge(iota_sem, 1)
            nc.vector.tensor_tensor(
                out=idx[:], in0=idx[:], in1=oob[:], op=mybir.AluOpType.add
            ).then_inc(idx_sem, 1)
            if prezero:
                for t in range(1, NT):
                    nc.vector.memset(data[t][:], 0.0).then_inc(zero_sem, 1)

    _program_cache[key] = nc
    return nc


def _keep_range(keep_token_ids: np.ndarray) -> tuple[int, int] | None:
    """If keep_token_ids is a contiguous integer range, return (lo, hi)."""
    k = np.asarray(keep_token_ids)
    if k.ndim != 1 or k.size == 0:
        return None
    lo = int(k.min())
    hi = int(k.max()) + 1
    if hi - lo == k.size and np.unique(k).size == k.size:
        return lo, hi
    return None


def kernel(input_embeddings, token_ids, keep_token_ids, _want_timing=False,
           _prezero=True):
    emb = np.ascontiguousarray(np.asarray(input_embeddings, dtype=np.float32))
    tok = np.ascontiguousarray(np.asarray(token_ids, dtype=np.int32))
    keep = np.asarray(keep_token_ids)
    assert emb.shape == (B, S, D) and tok.shape == (B, S)

    rng = _keep_range(keep)
    if rng is None or rng[0] != 0:
        # Keep-set is not arange(0, k) (not expected per spec): remap token
        # ids on the host so the device threshold compare still yields isin().
        tok = np.where(np.isin(tok, keep), np.int32(0), np.int32(1)).astype(np.int32)
        hi = 1
    else:
        hi = rng[1]

    if _want_timing:
        _want_timing = _install_ntff_hook()
    nc = _build_program(hi, prezero=_prezero, mode="scatter")
    in_maps = [{"emb": emb[b], "tok": tok[b]} for b in range(B)]
    res = run_bass_kernel_spmd(
        nc, in_maps, list(range(N_CORES)), trace=bool(_want_timing)
    )
    out = np.stack([np.asarray(res.results[b]["out"]) for b in range(B)], axis=0)
    if _want_timing:
        return out, res.exec_time_ns
    return out
